# revision 1
# baseline (speedup 1.0000x reference)
"""Trainium2 Bass kernel for a LoRA-augmented relu-gated MLP.

Math (per reference):
    y1 = x @ w_gate + b_gate + (x @ Ag) @ Bg
    y2 = x @ w_up   + b_up   + (x @ Au) @ Bu
    x3 = relu(y1) * y2
    y3 = x3 @ w_down + b_down + (x3 @ Ad) @ Bd

Strategy:
  * Host folds every LoRA pair into its base matrix (W_eff = W + A@B in
    float64, rounded to f32) so the device kernel is a plain gated MLP.
  * Data parallel over the 8 NeuronCores: 8192 tokens -> 1024 per core,
    every core holds the full (folded) weights.
  * All matmuls run as float32r (full-rate fp32 on the PE when the moving
    dim is >= 256).
  * Per core: x is transposed on-chip (PE transpose) to xT[k, m]; the MLP
    is computed in f-quarters: gate/up produce x3T stripes [128f, NT] in
    SBUF; the down projection consumes them as stationary operands and
    accumulates partial y3 straight into DRAM via accumulate-DMA.
    b_down enters through a rank-1 ones-matmul in the first quarter.
"""

import sys
import types

import numpy as np

# The trimmed container's `antenv` lacks `axon_hooks`; bass_utils imports it
# unconditionally when tracing is requested (e.g. BASS_TRACE=1). Provide the
# degraded no-hook module so tracing falls back gracefully instead of crashing.
try:
    import antenv.axon_hooks  # noqa: F401
except ImportError:
    _m = types.ModuleType("antenv.axon_hooks")
    _m._hook = None
    _m.set_axon_ntff_profile_hook = lambda h: setattr(_m, "_hook", h)
    _m.get_axon_ntff_profile_hook = lambda: _m._hook
    sys.modules["antenv.axon_hooks"] = _m

import concourse.bacc as bacc
import concourse.bass as bass
import concourse.mybir as mybir
import concourse.tile as tile
from concourse.bass_utils import run_bass_kernel_spmd
P = 128
F32 = mybir.dt.float32
F32R = mybir.dt.float32r
AF = mybir.ActivationFunctionType
ALU = mybir.AluOpType


class Cfg:
    def __init__(self, nt=1024, d=2048, f=8192, fq=4, n_cores=8):
        assert nt % P == 0 and d % P == 0 and f % P == 0
        self.NT = nt          # tokens per core
        self.D = d            # model dim
        self.F = f            # ffn dim
        self.KC = d // P      # contraction chunks for gate/up
        self.NF = f // P      # f-tiles
        self.FQ = fq          # f quarters (x3T resident per quarter)
        assert self.NF % fq == 0
        self.SQ = self.NF // fq
        self.MH = min(512, nt)          # moving-dim chunk for gate/up
        self.NMH = nt // self.MH
        self.DC = min(512, d)           # down-proj d chunk
        self.ND = d // self.DC
        self.NM = nt // P               # token chunks of 128
        self.MG = min(8, self.NM)       # psum group size for down-proj
        self.NMG = self.NM // self.MG
        self.N_CORES = n_cores


def build_bass(cfg: Cfg):
    """Builds the per-core Bass program (same program on all cores)."""
    c = cfg
    nc = bacc.Bacc("TRN2", target_bir_lowering=False, debug=False,
                   num_swdge_queues=4)

    xt = nc.dram_tensor("xt", [P, c.KC, c.NT], F32R, kind="ExternalInput")
    wg = nc.dram_tensor("wg", [c.NF, P, c.KC, P], F32R, kind="ExternalInput")
    wu = nc.dram_tensor("wu", [c.NF, P, c.KC, P], F32R, kind="ExternalInput")
    wd = nc.dram_tensor("wd", [c.NF, c.ND, P, c.DC], F32R, kind="ExternalInput")
    bg = nc.dram_tensor("bg", [P, c.NF], F32, kind="ExternalInput")
    bu = nc.dram_tensor("bu", [P, c.NF], F32, kind="ExternalInput")
    # cst row 0: ones (first 128 entries used), row 1: b_down
    cst = nc.dram_tensor("cst", [2, c.D], F32R, kind="ExternalInput")
    y = nc.dram_tensor("y", [c.NT, c.D], F32, kind="ExternalOutput")

    with tile.TileContext(nc) as tc:
        with (
            tc.tile_pool(name="consts", bufs=1) as consts,
            tc.tile_pool(name="wpool", bufs=4) as wpool,
            tc.tile_pool(name="wdpool", bufs=6) as wdpool,
            tc.tile_pool(name="xTp", bufs=1) as xTp,
            tc.tile_pool(name="x3p", bufs=1) as x3p,
            tc.tile_pool(name="actp", bufs=2) as actp,
            tc.tile_pool(name="outp", bufs=10) as outp,
            tc.tile_pool(name="pall", bufs=1, space="PSUM") as pall,
        ):
            bgt = consts.tile([P, c.NF], F32, name="bgt")
            nc.sync.dma_start(bgt, bg[:, :])
            but = consts.tile([P, c.NF], F32, name="but")
            nc.sync.dma_start(but, bu[:, :])
            ones = consts.tile([1, P], F32R, name="ones")
            nc.sync.dma_start(ones, cst[0:1, 0:P])
            bdr = consts.tile([1, c.D], F32R, name="bdr")
            nc.sync.dma_start(bdr, cst[1:2, :])

            # ---- load pre-transposed x: xT[kk, k_idx, m] ----
            # split by token-half so the first gate/up groups can start
            # as soon as the h=0 halves land
            xT = xTp.tile([P, c.KC, c.NT], F32R, name="xT")
            with tc.high_priority():
                for h in range(c.NMH):
                    msl = slice(h * c.MH, (h + 1) * c.MH)
                    for k in range(c.KC):
                        nc.sync.dma_start(xT[:, k, msl], xt[:, k, msl])

            DTAGS = ["p1", "p1", "p2", "p2", "pd0", "pd1", "pd2", "pd3"]
            for q in range(c.FQ):
                # ---- gate/up projections for this f-quarter ----
                x3 = [
                    x3p.tile([P, c.NT], F32R, tag=f"s{s}", name=f"x3_{q}_{s}")
                    for s in range(c.SQ)
                ]
                for s in range(c.SQ):
                    ft = q * c.SQ + s
                    wgt = wpool.tile([P, c.KC, P], F32R, tag="w", name=f"wg{ft}")
                    nc.sync.dma_start(wgt, wg[ft])
                    wut = wpool.tile([P, c.KC, P], F32R, tag="w", name=f"wu{ft}")
                    nc.sync.dma_start(wut, wu[ft])
                    for h in range(c.NMH):
                        msl = slice(h * c.MH, (h + 1) * c.MH)
                        p1 = pall.tile([P, c.MH], F32, tag="p1", bufs=2,
                                       name=f"p1_{ft}_{h}")
                        p2 = pall.tile([P, c.MH], F32, tag="p2", bufs=2,
                                       name=f"p2_{ft}_{h}")
                        for k in range(c.KC):
                            nc.tensor.matmul(
                                p1, wgt[:, k, :],
                                xT[:, k, msl],
                                start=(k == 0), stop=(k == c.KC - 1))
                        for k in range(c.KC):
                            nc.tensor.matmul(
                                p2, wut[:, k, :],
                                xT[:, k, msl],
                                start=(k == 0), stop=(k == c.KC - 1))
                        t1 = actp.tile([P, c.MH], F32, tag="t1", name=f"t1_{ft}_{h}")
                        nc.scalar.activation(t1, p1, AF.Relu, bias=bgt[:, ft:ft + 1])
                        # x3 = (p2 + b_up) * relu(p1 + b_gate)
                        nc.vector.scalar_tensor_tensor(
                            x3[s][:, msl], p2, but[:, ft:ft + 1], t1,
                            op0=ALU.add, op1=ALU.mult)
                # ---- down projection partials for this f-quarter ----
                for d in range(c.ND):
                    dsl = slice(d * c.DC, (d + 1) * c.DC)
                    for g in range(c.NMG):
                        pds = [
                            pall.tile([P, c.DC], F32, tag=DTAGS[j],
                                      bufs=2 if DTAGS[j] in ("p1", "p2") else 1,
                                      name=f"pd_{q}_{d}_{g}_{j}")
                            for j in range(c.MG)
                        ]
                        if q == 0:
                            # seed psum with b_down via rank-1 ones matmul
                            for j in range(c.MG):
                                nc.tensor.matmul(
                                    pds[j], ones[:, 0:P], bdr[:, dsl],
                                    start=True, stop=False)
                        for s in range(c.SQ):
                            wdt = wdpool.tile([P, c.DC], F32R, tag="wd",
                                              name=f"wd_{q}_{d}_{g}_{s}")
                            nc.sync.dma_start(wdt, wd[q * c.SQ + s, d])
                            for j in range(c.MG):
                                m = g * c.MG + j
                                nc.tensor.matmul(
                                    pds[j],
                                    x3[s][:, m * P:(m + 1) * P],
                                    wdt,
                                    start=(s == 0 and q != 0),
                                    stop=(s == c.SQ - 1))
                        for j in range(c.MG):
                            m = g * c.MG + j
                            ot = outp.tile([P, c.DC], F32, tag="ot",
                                           name=f"ot_{q}_{d}_{g}_{j}")
                            if j % 2 == 0:
                                nc.vector.tensor_copy(ot, pds[j])
                            else:
                                nc.scalar.copy(ot, pds[j])
                            if q == 0:
                                nc.sync.dma_start(
                                    y[m * P:(m + 1) * P, dsl], ot)
                            else:
                                nc.gpsimd.dma_start(
                                    y[m * P:(m + 1) * P, dsl], ot,
                                    accum_op=ALU.add)

    nc.compile()
    return nc


def _prep_weights(w, a, b):
    """Fold LoRA into base weight (float64 accumulate, f32 round)."""
    weff = (w.astype(np.float64) + a.astype(np.float64) @ b.astype(np.float64))
    return weff.astype(np.float32)


def prep_inputs(inputs, cfg: Cfg):
    c = cfg
    x = np.asarray(inputs["x1"], np.float32).reshape(-1, c.D)
    n_tok = x.shape[0]
    assert n_tok == c.NT * c.N_CORES
    wg_e = _prep_weights(np.asarray(inputs["w_gate"], np.float32),
                         np.asarray(inputs["w_gate_lora_a"], np.float32),
                         np.asarray(inputs["w_gate_lora_b"], np.float32))
    wu_e = _prep_weights(np.asarray(inputs["w_up"], np.float32),
                         np.asarray(inputs["w_up_lora_a"], np.float32),
                         np.asarray(inputs["w_up_lora_b"], np.float32))
    wd_e = _prep_weights(np.asarray(inputs["w_down"], np.float32),
                         np.asarray(inputs["w_down_lora_a"], np.float32),
                         np.asarray(inputs["w_down_lora_b"], np.float32))
    # W[k_idx*P+kk, ft*P+ff] -> [ft, kk, k_idx, ff]
    wg_t = np.ascontiguousarray(
        wg_e.reshape(c.KC, P, c.NF, P).transpose(2, 1, 0, 3))
    wu_t = np.ascontiguousarray(
        wu_e.reshape(c.KC, P, c.NF, P).transpose(2, 1, 0, 3))
    # Wd[ft*P+ff, d*DC+dd] -> [ft, d, ff, dd]
    wd_t = np.ascontiguousarray(
        wd_e.reshape(c.NF, P, c.ND, c.DC).transpose(0, 2, 1, 3))
    bg2 = np.ascontiguousarray(
        np.asarray(inputs["b_gate"], np.float32).reshape(c.NF, P).T)
    bu2 = np.ascontiguousarray(
        np.asarray(inputs["b_up"], np.float32).reshape(c.NF, P).T)
    cst = np.zeros((2, c.D), np.float32)
    cst[0, :] = 1.0
    cst[1, :] = np.asarray(inputs["b_down"], np.float32)
    in_maps = []
    for i in range(c.N_CORES):
        xs = x[i * c.NT:(i + 1) * c.NT]
        # [NT, D] -> [kk, k_idx, m]
        xt = np.ascontiguousarray(
            xs.T.reshape(c.KC, P, c.NT).transpose(1, 0, 2))
        in_maps.append({
            "xt": xt,
            "wg": wg_t, "wu": wu_t, "wd": wd_t,
            "bg": bg2, "bu": bu2, "cst": cst,
        })
    return in_maps


_CACHE = {}


def run(inputs, trace=False, trace_kwargs=None):
    cfg = Cfg()
    b, s, d = np.asarray(inputs["x1"]).shape
    in_maps = prep_inputs(inputs, cfg)
    key = "full"
    if key not in _CACHE:
        _CACHE[key] = build_bass(cfg)
    nc = _CACHE[key]
    res = run_bass_kernel_spmd(
        nc, in_maps, list(range(cfg.N_CORES)),
        trace=trace, **(trace_kwargs or {}))
    y = np.concatenate([res.results[i]["y"] for i in range(cfg.N_CORES)], axis=0)
    return y.reshape(b, s, d).astype(np.float32), res


def kernel(**inputs) -> np.ndarray:
    out, _ = run(inputs, trace=False)
    return out



# revision 2
# speedup vs baseline: 1.2613x; 1.2613x over previous
"""Trainium2 Bass kernel for a LoRA-augmented relu-gated MLP.

Math (per reference):
    y1 = x @ w_gate + b_gate + (x @ Ag) @ Bg
    y2 = x @ w_up   + b_up   + (x @ Au) @ Bu
    x3 = relu(y1) * y2
    y3 = x3 @ w_down + b_down + (x3 @ Ad) @ Bd

Strategy:
  * Host folds every LoRA pair into its base matrix (W_eff = W + A@B in
    float64) and rounds weights + activations to bf16 so the device kernel
    is a plain gated MLP running bf16 matmuls with f32 PSUM accumulation.
    bf16 stationary operands enable the PE's Fast Weight Load path, which
    hides the per-matmul LDWEIGHTS under the 512-column stream (fp32
    weights load at half rate and serialize ~60ns/matmul on top).
  * Data parallel over the 8 NeuronCores: 8192 tokens -> 1024 per core,
    every core holds the full (folded) weights.
  * Per core the MLP is computed in f-quarters: gate/up produce x3T
    stripes [128f, NT] (bf16) in SBUF; the down projection consumes them
    as stationary operands and accumulates partial y3 into an SBUF-resident
    f32 accumulator; b_down is added on the first quarter's eviction. The
    last quarter streams finished y chunks straight to DRAM.
"""

import sys
import types

import numpy as np

# The trimmed container's `antenv` lacks `axon_hooks`; bass_utils imports it
# unconditionally when tracing is requested (e.g. BASS_TRACE=1). Provide the
# degraded no-hook module so tracing falls back gracefully instead of crashing.
try:
    import antenv.axon_hooks  # noqa: F401
except ImportError:
    _m = types.ModuleType("antenv.axon_hooks")
    _m._hook = None
    _m.set_axon_ntff_profile_hook = lambda h: setattr(_m, "_hook", h)
    _m.get_axon_ntff_profile_hook = lambda: _m._hook
    sys.modules["antenv.axon_hooks"] = _m

import concourse.bacc as bacc
import concourse.bass as bass
import concourse.mybir as mybir
import concourse.tile as tile
from concourse.bass_utils import run_bass_kernel_spmd

P = 128
F32 = mybir.dt.float32
BF16 = mybir.dt.bfloat16
NP_BF16 = mybir.dt.np(mybir.dt.bfloat16)
AF = mybir.ActivationFunctionType
ALU = mybir.AluOpType


class Cfg:
    def __init__(self, nt=1024, d=2048, f=8192, fq=4, n_cores=8):
        assert nt % P == 0 and d % P == 0 and f % P == 0
        self.NT = nt          # tokens per core
        self.D = d            # model dim
        self.F = f            # ffn dim
        self.KC = d // P      # contraction chunks for gate/up
        self.NF = f // P      # f-tiles
        self.FQ = fq          # f quarters (x3T resident per quarter)
        assert self.NF % fq == 0
        self.SQ = self.NF // fq
        self.MH = min(512, nt)          # moving-dim chunk for gate/up
        self.NMH = nt // self.MH
        self.DC = min(512, d)           # down-proj d chunk
        self.ND = d // self.DC
        self.NM = nt // P               # token chunks of 128
        self.MG = 4                     # psum group size for down-proj
        self.NMG = self.NM // self.MG
        self.N_CORES = n_cores


def build_bass(cfg: Cfg):
    """Builds the per-core Bass program (same program on all cores)."""
    c = cfg
    nc = bacc.Bacc("TRN2", target_bir_lowering=False, debug=False,
                   num_swdge_queues=4)

    xt = nc.dram_tensor("xt", [P, c.KC, c.NT], BF16, kind="ExternalInput")
    wg = nc.dram_tensor("wg", [c.NF, P, c.KC, P], BF16, kind="ExternalInput")
    wu = nc.dram_tensor("wu", [c.NF, P, c.KC, P], BF16, kind="ExternalInput")
    wd = nc.dram_tensor("wd", [c.NF, c.ND, P, c.DC], BF16, kind="ExternalInput")
    bg = nc.dram_tensor("bg", [P, c.NF], F32, kind="ExternalInput")
    bu = nc.dram_tensor("bu", [P, c.NF], F32, kind="ExternalInput")
    bd = nc.dram_tensor("bd", [P, c.D], F32, kind="ExternalInput")
    y = nc.dram_tensor("y", [c.NT, c.D], F32, kind="ExternalOutput")

    with tile.TileContext(nc) as tc:
        with (
            tc.tile_pool(name="consts", bufs=1) as consts,
            tc.tile_pool(name="wpool", bufs=4) as wpool,
            tc.tile_pool(name="wdpool", bufs=6) as wdpool,
            tc.tile_pool(name="xTp", bufs=1) as xTp,
            tc.tile_pool(name="x3p", bufs=1) as x3p,
            tc.tile_pool(name="yp", bufs=1) as yp,
            tc.tile_pool(name="actp", bufs=2) as actp,
            tc.tile_pool(name="outp", bufs=6) as outp,
            tc.tile_pool(name="pall", bufs=1, space="PSUM") as pall,
        ):
            def load_w(ft):
                wgt = wpool.tile([P, c.KC, P], BF16, tag="w", name=f"wg{ft}")
                nc.sync.dma_start(wgt, wg[ft])
                wut = wpool.tile([P, c.KC, P], BF16, tag="w", name=f"wu{ft}")
                nc.sync.dma_start(wut, wu[ft])
                return wgt, wut

            # first f-tiles' weights come before the bulk x load so the PE
            # can start as soon as the first x chunk lands
            with tc.high_priority():
                pend = {0: load_w(0), 1: load_w(1)}
                bgt = consts.tile([P, c.NF], F32, name="bgt")
                nc.sync.dma_start(bgt, bg[:, :])
                but = consts.tile([P, c.NF], F32, name="but")
                nc.sync.dma_start(but, bu[:, :])
                bdf = consts.tile([P, c.D], F32, name="bdf")
                nc.sync.dma_start(bdf, bd[:, :])

            # ---- load pre-transposed x: xT[kk, k_idx, m] ----
            xT = xTp.tile([P, c.KC, c.NT], BF16, name="xT")
            with tc.high_priority():
                for h in range(c.NMH):
                    msl = slice(h * c.MH, (h + 1) * c.MH)
                    for k in range(c.KC):
                        nc.sync.dma_start(xT[:, k, msl], xt[:, k, msl])

            # y accumulator, SBUF-resident across the 4 f-quarters
            yacc = yp.tile([P, c.NM, c.D], F32, name="yacc")

            DTAGS = ["p1", "p2", "pd0", "pd1"]
            for q in range(c.FQ):
                # ---- gate/up projections for this f-quarter ----
                x3 = [
                    x3p.tile([P, c.NT], BF16, tag=f"s{s}", name=f"x3_{q}_{s}")
                    for s in range(c.SQ)
                ]
                for s in range(c.SQ):
                    ft = q * c.SQ + s
                    wgt, wut = pend.pop(ft) if ft in pend else load_w(ft)
                    if ft + 2 < c.NF and ft + 2 not in pend:
                        pend[ft + 2] = load_w(ft + 2)
                    for h in range(c.NMH):
                        msl = slice(h * c.MH, (h + 1) * c.MH)
                        p1 = pall.tile([P, c.MH], F32, tag="p1", bufs=2,
                                       name=f"p1_{ft}_{h}")
                        p2 = pall.tile([P, c.MH], F32, tag="p2", bufs=2,
                                       name=f"p2_{ft}_{h}")
                        for k in range(c.KC):
                            nc.tensor.matmul(
                                p1, wgt[:, k, :],
                                xT[:, k, msl],
                                start=(k == 0), stop=(k == c.KC - 1))
                        for k in range(c.KC):
                            nc.tensor.matmul(
                                p2, wut[:, k, :],
                                xT[:, k, msl],
                                start=(k == 0), stop=(k == c.KC - 1))
                        t1 = actp.tile([P, c.MH], F32, tag="t1", name=f"t1_{ft}_{h}")
                        nc.scalar.activation(t1, p1, AF.Relu, bias=bgt[:, ft:ft + 1])
                        # x3 = (p2 + b_up) * relu(p1 + b_gate)
                        nc.vector.scalar_tensor_tensor(
                            x3[s][:, msl], p2, but[:, ft:ft + 1], t1,
                            op0=ALU.add, op1=ALU.mult)
                # ---- down projection partials for this f-quarter ----
                for d in range(c.ND):
                    dsl = slice(d * c.DC, (d + 1) * c.DC)
                    for g in range(c.NMG):
                        pds = [
                            pall.tile([P, c.DC], F32, tag=DTAGS[j], bufs=2,
                                      name=f"pd_{q}_{d}_{g}_{j}")
                            for j in range(c.MG)
                        ]
                        for s in range(c.SQ):
                            wdt = wdpool.tile([P, c.DC], BF16, tag="wd",
                                              name=f"wd_{q}_{d}_{g}_{s}")
                            nc.sync.dma_start(wdt, wd[q * c.SQ + s, d])
                            for j in range(c.MG):
                                m = g * c.MG + j
                                nc.tensor.matmul(
                                    pds[j],
                                    x3[s][:, m * P:(m + 1) * P],
                                    wdt,
                                    start=(s == 0),
                                    stop=(s == c.SQ - 1))
                        for j in range(c.MG):
                            m = g * c.MG + j
                            if q == 0:
                                # seed with b_down on the first partial
                                nc.vector.tensor_add(
                                    yacc[:, m, dsl], pds[j], bdf[:, dsl])
                            elif q < c.FQ - 1:
                                nc.vector.tensor_add(
                                    yacc[:, m, dsl], pds[j], yacc[:, m, dsl])
                            else:
                                ot = outp.tile([P, c.DC], F32, tag="ot",
                                               name=f"ot_{d}_{g}_{j}")
                                nc.vector.tensor_add(
                                    ot, pds[j], yacc[:, m, dsl])
                                nc.sync.dma_start(
                                    y[m * P:(m + 1) * P, dsl], ot)

    nc.compile()
    return nc


def _prep_weights(w, a, b):
    """Fold LoRA into base weight (float64 accumulate, bf16 round)."""
    weff = (w.astype(np.float64) + a.astype(np.float64) @ b.astype(np.float64))
    return weff.astype(np.float32)


def prep_inputs(inputs, cfg: Cfg):
    c = cfg
    x = np.asarray(inputs["x1"], np.float32).reshape(-1, c.D)
    n_tok = x.shape[0]
    assert n_tok == c.NT * c.N_CORES
    wg_e = _prep_weights(np.asarray(inputs["w_gate"], np.float32),
                         np.asarray(inputs["w_gate_lora_a"], np.float32),
                         np.asarray(inputs["w_gate_lora_b"], np.float32))
    wu_e = _prep_weights(np.asarray(inputs["w_up"], np.float32),
                         np.asarray(inputs["w_up_lora_a"], np.float32),
                         np.asarray(inputs["w_up_lora_b"], np.float32))
    wd_e = _prep_weights(np.asarray(inputs["w_down"], np.float32),
                         np.asarray(inputs["w_down_lora_a"], np.float32),
                         np.asarray(inputs["w_down_lora_b"], np.float32))
    # W[k_idx*P+kk, ft*P+ff] -> [ft, kk, k_idx, ff]
    wg_t = np.ascontiguousarray(
        wg_e.reshape(c.KC, P, c.NF, P).transpose(2, 1, 0, 3)).astype(NP_BF16)
    wu_t = np.ascontiguousarray(
        wu_e.reshape(c.KC, P, c.NF, P).transpose(2, 1, 0, 3)).astype(NP_BF16)
    # Wd[ft*P+ff, d*DC+dd] -> [ft, d, ff, dd]
    wd_t = np.ascontiguousarray(
        wd_e.reshape(c.NF, P, c.ND, c.DC).transpose(0, 2, 1, 3)).astype(NP_BF16)
    bg2 = np.ascontiguousarray(
        np.asarray(inputs["b_gate"], np.float32).reshape(c.NF, P).T)
    bu2 = np.ascontiguousarray(
        np.asarray(inputs["b_up"], np.float32).reshape(c.NF, P).T)
    bdf = np.ascontiguousarray(np.broadcast_to(
        np.asarray(inputs["b_down"], np.float32), (P, c.D)))
    in_maps = []
    for i in range(c.N_CORES):
        xs = x[i * c.NT:(i + 1) * c.NT]
        # [NT, D] -> [kk, k_idx, m]
        xt = np.ascontiguousarray(
            xs.T.reshape(c.KC, P, c.NT).transpose(1, 0, 2)).astype(NP_BF16)
        in_maps.append({
            "xt": xt,
            "wg": wg_t, "wu": wu_t, "wd": wd_t,
            "bg": bg2, "bu": bu2, "bd": bdf,
        })
    return in_maps


_CACHE = {}


def run(inputs, trace=False, trace_kwargs=None):
    cfg = Cfg()
    b, s, d = np.asarray(inputs["x1"]).shape
    in_maps = prep_inputs(inputs, cfg)
    key = "full"
    if key not in _CACHE:
        _CACHE[key] = build_bass(cfg)
    nc = _CACHE[key]
    res = run_bass_kernel_spmd(
        nc, in_maps, list(range(cfg.N_CORES)),
        trace=trace, **(trace_kwargs or {}))
    y = np.concatenate([res.results[i]["y"] for i in range(cfg.N_CORES)], axis=0)
    return y.reshape(b, s, d).astype(np.float32), res


def kernel(**inputs) -> np.ndarray:
    out, _ = run(inputs, trace=False)
    return out


# revision 7
# speedup vs baseline: 1.2770x; 1.0124x over previous
"""Trainium2 Bass kernel for a LoRA-augmented relu-gated MLP.

Math (per reference):
    y1 = x @ w_gate + b_gate + (x @ Ag) @ Bg
    y2 = x @ w_up   + b_up   + (x @ Au) @ Bu
    x3 = relu(y1) * y2
    y3 = x3 @ w_down + b_down + (x3 @ Ad) @ Bd

Strategy:
  * Host folds every LoRA pair into its base matrix (W_eff = W + A@B in
    float64) and rounds weights + activations to bf16 so the device kernel
    is a plain gated MLP running bf16 matmuls with f32 PSUM accumulation.
    bf16 stationary operands enable the PE's Fast Weight Load path, which
    hides the per-matmul LDWEIGHTS under the 512-column stream (fp32
    weights load at half rate and serialize ~60ns/matmul on top).
  * Data parallel over the 8 NeuronCores: 8192 tokens -> 1024 per core,
    every core holds the full (folded) weights.
  * Per core the MLP is computed in f-quarters: gate/up produce x3T
    stripes [128f, NT] (bf16) in SBUF; the down projection consumes them
    as stationary operands and accumulates partial y3 into an SBUF-resident
    f32 accumulator; b_down is added on the first quarter's eviction. The
    last quarter streams finished y chunks straight to DRAM.
"""

import sys
import types

import numpy as np

# The trimmed container's `antenv` lacks `axon_hooks`; bass_utils imports it
# unconditionally when tracing is requested (e.g. BASS_TRACE=1). Provide the
# degraded no-hook module so tracing falls back gracefully instead of crashing.
try:
    import antenv.axon_hooks  # noqa: F401
except ImportError:
    _m = types.ModuleType("antenv.axon_hooks")
    _m._hook = None
    _m.set_axon_ntff_profile_hook = lambda h: setattr(_m, "_hook", h)
    _m.get_axon_ntff_profile_hook = lambda: _m._hook
    sys.modules["antenv.axon_hooks"] = _m

import concourse.bacc as bacc
import concourse.bass as bass
import concourse.mybir as mybir
import concourse.tile as tile
from concourse.bass_utils import run_bass_kernel_spmd

P = 128
F32 = mybir.dt.float32
BF16 = mybir.dt.bfloat16
NP_BF16 = mybir.dt.np(mybir.dt.bfloat16)
AF = mybir.ActivationFunctionType
ALU = mybir.AluOpType


class Cfg:
    def __init__(self, nt=1024, d=2048, f=8192, fq=4, n_cores=8):
        assert nt % P == 0 and d % P == 0 and f % P == 0
        self.NT = nt          # tokens per core
        self.D = d            # model dim
        self.F = f            # ffn dim
        self.KC = d // P      # contraction chunks for gate/up
        self.NF = f // P      # f-tiles
        self.FQ = fq          # f quarters (x3T resident per quarter)
        assert self.NF % fq == 0
        self.SQ = self.NF // fq
        self.MH = min(512, nt)          # moving-dim chunk for gate/up
        self.NMH = nt // self.MH
        self.DC = min(512, d)           # down-proj d chunk
        self.ND = d // self.DC
        self.NM = nt // P               # token chunks of 128
        self.MG = 4                     # psum group size for down-proj
        self.NMG = self.NM // self.MG
        self.N_CORES = n_cores


def build_bass(cfg: Cfg):
    """Builds the per-core Bass program (same program on all cores)."""
    c = cfg
    nc = bacc.Bacc("TRN2", target_bir_lowering=False, debug=False,
                   num_swdge_queues=4)

    xt = nc.dram_tensor("xt", [P, c.KC, c.NT], BF16, kind="ExternalInput")
    wg = nc.dram_tensor("wg", [c.NF, P, c.KC, P], BF16, kind="ExternalInput")
    wu = nc.dram_tensor("wu", [c.NF, P, c.KC, P], BF16, kind="ExternalInput")
    wd = nc.dram_tensor("wd", [c.ND, P, c.NF, c.DC], BF16, kind="ExternalInput")
    bg = nc.dram_tensor("bg", [P, c.NF], F32, kind="ExternalInput")
    bu = nc.dram_tensor("bu", [P, c.NF], F32, kind="ExternalInput")
    bd = nc.dram_tensor("bd", [P, c.D], F32, kind="ExternalInput")
    y = nc.dram_tensor("y", [c.NT, c.D], F32, kind="ExternalOutput")

    with tile.TileContext(nc) as tc:
        with (
            tc.tile_pool(name="consts", bufs=1) as consts,
            tc.tile_pool(name="wpool", bufs=4) as wpool,
            tc.tile_pool(name="wdpool", bufs=3) as wdpool,
            tc.tile_pool(name="xTp", bufs=1) as xTp,
            tc.tile_pool(name="x3p", bufs=1) as x3p,
            tc.tile_pool(name="yp", bufs=1) as yp,
            tc.tile_pool(name="actp", bufs=2) as actp,
            tc.tile_pool(name="outp", bufs=6) as outp,
            tc.tile_pool(name="pall", bufs=1, space="PSUM") as pall,
        ):
            # weight stream + y writeback ride SWDGE (gpsimd) queues; the
            # SP HWDGE ring is reserved for xT and the down-proj wd batches
            # so neither stream head-of-line-blocks the other.
            def load_w(ft):
                wgt = wpool.tile([P, c.KC, P], BF16, tag="w", name=f"wg{ft}")
                nc.gpsimd.dma_start(wgt, wg[ft])
                wut = wpool.tile([P, c.KC, P], BF16, tag="w", name=f"wu{ft}")
                nc.gpsimd.dma_start(wut, wu[ft])
                return wgt, wut

            # first f-tiles' weights come before the bulk x load so the PE
            # can start as soon as the first x chunk lands
            with tc.high_priority():
                pend = {0: load_w(0), 1: load_w(1)}
                bgt = consts.tile([P, c.NF], F32, name="bgt")
                nc.sync.dma_start(bgt, bg[:, :])
                but = consts.tile([P, c.NF], F32, name="but")
                nc.sync.dma_start(but, bu[:, :])

            # ---- load pre-transposed x: xT[kk, k_idx, m] ----
            # batches of 8 k-chunks: few enough DMAs that the SP ring never
            # serializes, granular enough that the PE starts within ~4us
            xT = xTp.tile([P, c.KC, c.NT], BF16, name="xT")
            KB = 8
            with tc.high_priority():
                for h in range(c.NMH):
                    msl = slice(h * c.MH, (h + 1) * c.MH)
                    for k0 in range(0, c.KC, KB):
                        nc.sync.dma_start(xT[:, k0:k0 + KB, msl],
                                          xt[:, k0:k0 + KB, msl])
            bdf = consts.tile([P, c.D], F32, name="bdf")
            nc.gpsimd.dma_start(bdf, bd[:, :])

            # y accumulator, SBUF-resident across the 4 f-quarters
            yacc = yp.tile([P, c.NM, c.D], F32, name="yacc")

            DTAGS = ["p1", "p2", "pd0", "pd1"]
            for q in range(c.FQ):
                # ---- gate/up projections for this f-quarter ----
                x3 = [
                    x3p.tile([P, c.NT], BF16, tag=f"s{s}", name=f"x3_{q}_{s}")
                    for s in range(c.SQ)
                ]
                for s in range(c.SQ):
                    ft = q * c.SQ + s
                    wgt, wut = pend.pop(ft) if ft in pend else load_w(ft)
                    if ft + 2 < c.NF and ft + 2 not in pend:
                        pend[ft + 2] = load_w(ft + 2)
                    for h in range(c.NMH):
                        msl = slice(h * c.MH, (h + 1) * c.MH)
                        p1 = pall.tile([P, c.MH], F32, tag="p1", bufs=2,
                                       name=f"p1_{ft}_{h}")
                        p2 = pall.tile([P, c.MH], F32, tag="p2", bufs=2,
                                       name=f"p2_{ft}_{h}")
                        for k in range(c.KC):
                            nc.tensor.matmul(
                                p1, wgt[:, k, :],
                                xT[:, k, msl],
                                start=(k == 0), stop=(k == c.KC - 1))
                        for k in range(c.KC):
                            nc.tensor.matmul(
                                p2, wut[:, k, :],
                                xT[:, k, msl],
                                start=(k == 0), stop=(k == c.KC - 1))
                        t1 = actp.tile([P, c.MH], F32, tag="t1", name=f"t1_{ft}_{h}")
                        nc.scalar.activation(t1, p1, AF.Relu, bias=bgt[:, ft:ft + 1])
                        # x3 = (p2 + b_up) * relu(p1 + b_gate)
                        nc.vector.scalar_tensor_tensor(
                            x3[s][:, msl], p2, but[:, ft:ft + 1], t1,
                            op0=ALU.add, op1=ALU.mult)
                # ---- down projection partials for this f-quarter ----
                SB = 8  # wd stripes per DMA batch
                for d in range(c.ND):
                    dsl = slice(d * c.DC, (d + 1) * c.DC)
                    for g in range(c.NMG):
                        pds = [
                            pall.tile([P, c.DC], F32, tag=DTAGS[j], bufs=2,
                                      name=f"pd_{q}_{d}_{g}_{j}")
                            for j in range(c.MG)
                        ]
                        for s0 in range(0, c.SQ, SB):
                            wdt = wdpool.tile([P, SB, c.DC], BF16, tag="wd",
                                              name=f"wd_{q}_{d}_{g}_{s0}")
                            f0 = q * c.SQ + s0
                            nc.sync.dma_start(wdt, wd[d][:, f0:f0 + SB, :])
                            for si in range(SB):
                                s = s0 + si
                                for j in range(c.MG):
                                    m = g * c.MG + j
                                    nc.tensor.matmul(
                                        pds[j],
                                        x3[s][:, m * P:(m + 1) * P],
                                        wdt[:, si, :],
                                        start=(s == 0),
                                        stop=(s == c.SQ - 1))
                        for j in range(c.MG):
                            m = g * c.MG + j
                            if q == 0:
                                # seed with b_down on the first partial
                                nc.vector.tensor_add(
                                    yacc[:, m, dsl], pds[j], bdf[:, dsl])
                            elif q < c.FQ - 1:
                                nc.vector.tensor_add(
                                    yacc[:, m, dsl], pds[j], yacc[:, m, dsl])
                            else:
                                ot = outp.tile([P, c.DC], F32, tag="ot",
                                               name=f"ot_{d}_{g}_{j}")
                                nc.vector.tensor_add(
                                    ot, pds[j], yacc[:, m, dsl])
                                nc.gpsimd.dma_start(
                                    y[m * P:(m + 1) * P, dsl], ot)

    nc.compile()
    return nc


def _prep_weights(w, a, b):
    """Fold LoRA into base weight (float64 accumulate, bf16 round)."""
    weff = (w.astype(np.float64) + a.astype(np.float64) @ b.astype(np.float64))
    return weff.astype(np.float32)


def prep_inputs(inputs, cfg: Cfg):
    c = cfg
    x = np.asarray(inputs["x1"], np.float32).reshape(-1, c.D)
    n_tok = x.shape[0]
    assert n_tok == c.NT * c.N_CORES
    wg_e = _prep_weights(np.asarray(inputs["w_gate"], np.float32),
                         np.asarray(inputs["w_gate_lora_a"], np.float32),
                         np.asarray(inputs["w_gate_lora_b"], np.float32))
    wu_e = _prep_weights(np.asarray(inputs["w_up"], np.float32),
                         np.asarray(inputs["w_up_lora_a"], np.float32),
                         np.asarray(inputs["w_up_lora_b"], np.float32))
    wd_e = _prep_weights(np.asarray(inputs["w_down"], np.float32),
                         np.asarray(inputs["w_down_lora_a"], np.float32),
                         np.asarray(inputs["w_down_lora_b"], np.float32))
    # W[k_idx*P+kk, ft*P+ff] -> [ft, kk, k_idx, ff]
    wg_t = np.ascontiguousarray(
        wg_e.reshape(c.KC, P, c.NF, P).transpose(2, 1, 0, 3)).astype(NP_BF16)
    wu_t = np.ascontiguousarray(
        wu_e.reshape(c.KC, P, c.NF, P).transpose(2, 1, 0, 3)).astype(NP_BF16)
    # Wd[ft*P+ff, d*DC+dd] -> [d, ff, ft, dd]
    wd_t = np.ascontiguousarray(
        wd_e.reshape(c.NF, P, c.ND, c.DC).transpose(2, 1, 0, 3)).astype(NP_BF16)
    bg2 = np.ascontiguousarray(
        np.asarray(inputs["b_gate"], np.float32).reshape(c.NF, P).T)
    bu2 = np.ascontiguousarray(
        np.asarray(inputs["b_up"], np.float32).reshape(c.NF, P).T)
    bdf = np.ascontiguousarray(np.broadcast_to(
        np.asarray(inputs["b_down"], np.float32), (P, c.D)))
    in_maps = []
    for i in range(c.N_CORES):
        xs = x[i * c.NT:(i + 1) * c.NT]
        # [NT, D] -> [kk, k_idx, m]
        xt = np.ascontiguousarray(
            xs.T.reshape(c.KC, P, c.NT).transpose(1, 0, 2)).astype(NP_BF16)
        in_maps.append({
            "xt": xt,
            "wg": wg_t, "wu": wu_t, "wd": wd_t,
            "bg": bg2, "bu": bu2, "bd": bdf,
        })
    return in_maps


_CACHE = {}


def run(inputs, trace=False, trace_kwargs=None):
    cfg = Cfg()
    b, s, d = np.asarray(inputs["x1"]).shape
    in_maps = prep_inputs(inputs, cfg)
    key = "full"
    if key not in _CACHE:
        _CACHE[key] = build_bass(cfg)
    nc = _CACHE[key]
    res = run_bass_kernel_spmd(
        nc, in_maps, list(range(cfg.N_CORES)),
        trace=trace, **(trace_kwargs or {}))
    y = np.concatenate([res.results[i]["y"] for i in range(cfg.N_CORES)], axis=0)
    return y.reshape(b, s, d).astype(np.float32), res


def kernel(**inputs) -> np.ndarray:
    out, _ = run(inputs, trace=False)
    return out


# revision 9
# speedup vs baseline: 1.2801x; 1.0024x over previous
"""Trainium2 Bass kernel for a LoRA-augmented relu-gated MLP.

Math (per reference):
    y1 = x @ w_gate + b_gate + (x @ Ag) @ Bg
    y2 = x @ w_up   + b_up   + (x @ Au) @ Bu
    x3 = relu(y1) * y2
    y3 = x3 @ w_down + b_down + (x3 @ Ad) @ Bd

Strategy:
  * Host folds every LoRA pair into its base matrix (W_eff = W + A@B in
    float64) and rounds weights + activations to bf16 so the device kernel
    is a plain gated MLP running bf16 matmuls with f32 PSUM accumulation.
    bf16 stationary operands enable the PE's Fast Weight Load path, which
    hides the per-matmul LDWEIGHTS under the 512-column stream (fp32
    weights load at half rate and serialize ~60ns/matmul on top).
  * Data parallel over the 8 NeuronCores: 8192 tokens -> 1024 per core,
    every core holds the full (folded) weights.
  * Per core the MLP is computed in f-quarters: gate/up produce x3T
    stripes [128f, NT] (bf16) in SBUF; the down projection consumes them
    as stationary operands and accumulates partial y3 into an SBUF-resident
    f32 accumulator; b_down is added on the first quarter's eviction. The
    last quarter streams finished y chunks straight to DRAM.
"""

import sys
import types

import numpy as np

# The trimmed container's `antenv` lacks `axon_hooks`; bass_utils imports it
# unconditionally when tracing is requested (e.g. BASS_TRACE=1). Provide the
# degraded no-hook module so tracing falls back gracefully instead of crashing.
try:
    import antenv.axon_hooks  # noqa: F401
except ImportError:
    _m = types.ModuleType("antenv.axon_hooks")
    _m._hook = None
    _m.set_axon_ntff_profile_hook = lambda h: setattr(_m, "_hook", h)
    _m.get_axon_ntff_profile_hook = lambda: _m._hook
    sys.modules["antenv.axon_hooks"] = _m

import concourse.bacc as bacc
import concourse.bass as bass
import concourse.mybir as mybir
import concourse.tile as tile
from concourse.bass_utils import run_bass_kernel_spmd

P = 128
F32 = mybir.dt.float32
BF16 = mybir.dt.bfloat16
NP_BF16 = mybir.dt.np(mybir.dt.bfloat16)
AF = mybir.ActivationFunctionType
ALU = mybir.AluOpType


class Cfg:
    def __init__(self, nt=1024, d=2048, f=8192, fq=4, n_cores=8):
        assert nt % P == 0 and d % P == 0 and f % P == 0
        self.NT = nt          # tokens per core
        self.D = d            # model dim
        self.F = f            # ffn dim
        self.KC = d // P      # contraction chunks for gate/up
        self.NF = f // P      # f-tiles
        self.FQ = fq          # f quarters (x3T resident per quarter)
        assert self.NF % fq == 0
        self.SQ = self.NF // fq
        self.MH = min(512, nt)          # moving-dim chunk for gate/up
        self.NMH = nt // self.MH
        self.DC = min(512, d)           # down-proj d chunk
        self.ND = d // self.DC
        self.NM = nt // P               # token chunks of 128
        self.MG = 4                     # psum group size for down-proj
        self.NMG = self.NM // self.MG
        self.N_CORES = n_cores


def build_bass(cfg: Cfg):
    """Builds the per-core Bass program (same program on all cores)."""
    c = cfg
    nc = bacc.Bacc("TRN2", target_bir_lowering=False, debug=False,
                   num_swdge_queues=4)

    xt = nc.dram_tensor("xt", [P, c.KC, c.NT], BF16, kind="ExternalInput")
    wg = nc.dram_tensor("wg", [c.NF, P, c.KC, P], BF16, kind="ExternalInput")
    wu = nc.dram_tensor("wu", [c.NF, P, c.KC, P], BF16, kind="ExternalInput")
    wd = nc.dram_tensor("wd", [c.ND, P, c.NF, c.DC], BF16, kind="ExternalInput")
    bg = nc.dram_tensor("bg", [P, c.NF], F32, kind="ExternalInput")
    bu = nc.dram_tensor("bu", [P, c.NF], F32, kind="ExternalInput")
    bd = nc.dram_tensor("bd", [P, c.D], F32, kind="ExternalInput")
    y = nc.dram_tensor("y", [c.NT, c.D], F32, kind="ExternalOutput")

    with tile.TileContext(nc) as tc:
        with (
            tc.tile_pool(name="consts", bufs=1) as consts,
            tc.tile_pool(name="wpool", bufs=4) as wpool,
            tc.tile_pool(name="wdpool", bufs=3) as wdpool,
            tc.tile_pool(name="xTp", bufs=1) as xTp,
            tc.tile_pool(name="x3p", bufs=1) as x3p,
            tc.tile_pool(name="yp", bufs=1) as yp,
            tc.tile_pool(name="actp", bufs=2) as actp,
            tc.tile_pool(name="outp", bufs=6) as outp,
            tc.tile_pool(name="pall", bufs=1, space="PSUM") as pall,
        ):
            # weight stream + y writeback ride SWDGE (gpsimd) queues; the
            # SP HWDGE ring is reserved for xT and the down-proj wd batches
            # so neither stream head-of-line-blocks the other.
            def load_w(ft):
                wgt = wpool.tile([P, c.KC, P], BF16, tag="w", name=f"wg{ft}")
                nc.gpsimd.dma_start(wgt, wg[ft])
                wut = wpool.tile([P, c.KC, P], BF16, tag="w", name=f"wu{ft}")
                nc.gpsimd.dma_start(wut, wu[ft])
                return wgt, wut

            # first f-tiles' weights come before the bulk x load so the PE
            # can start as soon as the first x chunk lands
            with tc.high_priority():
                pend = {0: load_w(0), 1: load_w(1)}
                bgt = consts.tile([P, c.NF], F32, name="bgt")
                nc.sync.dma_start(bgt, bg[:, :])
                but = consts.tile([P, c.NF], F32, name="but")
                nc.sync.dma_start(but, bu[:, :])

            # ---- load pre-transposed x: xT[kk, k_idx, m] ----
            # staircase batch sizes: small first batches so the PE starts
            # (and the HAM clock warms) as early as possible, large later
            # batches so the SP ring never serializes on descriptor count
            xT = xTp.tile([P, c.KC, c.NT], BF16, name="xT")
            ladders = {0: [2, 2, 4, 8], 1: [8, 8]}
            with tc.high_priority():
                for h in range(c.NMH):
                    msl = slice(h * c.MH, (h + 1) * c.MH)
                    k0 = 0
                    for kb in ladders.get(h, [c.KC]):
                        nc.sync.dma_start(xT[:, k0:k0 + kb, msl],
                                          xt[:, k0:k0 + kb, msl])
                        k0 += kb
            bdf = consts.tile([P, c.D], F32, name="bdf")
            nc.gpsimd.dma_start(bdf, bd[:, :])

            # y accumulator, SBUF-resident across the 4 f-quarters
            yacc = yp.tile([P, c.NM, c.D], F32, name="yacc")

            DTAGS = ["p1", "p2", "pd0", "pd1"]
            for q in range(c.FQ):
                # ---- gate/up projections for this f-quarter ----
                x3 = [
                    x3p.tile([P, c.NT], BF16, tag=f"s{s}", name=f"x3_{q}_{s}")
                    for s in range(c.SQ)
                ]
                for s in range(c.SQ):
                    ft = q * c.SQ + s
                    wgt, wut = pend.pop(ft) if ft in pend else load_w(ft)
                    if ft + 2 < c.NF and ft + 2 not in pend:
                        pend[ft + 2] = load_w(ft + 2)
                    for h in range(c.NMH):
                        msl = slice(h * c.MH, (h + 1) * c.MH)
                        p1 = pall.tile([P, c.MH], F32, tag="p1", bufs=2,
                                       name=f"p1_{ft}_{h}")
                        p2 = pall.tile([P, c.MH], F32, tag="p2", bufs=2,
                                       name=f"p2_{ft}_{h}")
                        for k in range(c.KC):
                            nc.tensor.matmul(
                                p1, wgt[:, k, :],
                                xT[:, k, msl],
                                start=(k == 0), stop=(k == c.KC - 1))
                        for k in range(c.KC):
                            nc.tensor.matmul(
                                p2, wut[:, k, :],
                                xT[:, k, msl],
                                start=(k == 0), stop=(k == c.KC - 1))
                        t1 = actp.tile([P, c.MH], F32, tag="t1", name=f"t1_{ft}_{h}")
                        nc.scalar.activation(t1, p1, AF.Relu, bias=bgt[:, ft:ft + 1])
                        # x3 = (p2 + b_up) * relu(p1 + b_gate)
                        nc.vector.scalar_tensor_tensor(
                            x3[s][:, msl], p2, but[:, ft:ft + 1], t1,
                            op0=ALU.add, op1=ALU.mult)
                # ---- down projection partials for this f-quarter ----
                SB = 8  # wd stripes per DMA batch
                for d in range(c.ND):
                    dsl = slice(d * c.DC, (d + 1) * c.DC)
                    for g in range(c.NMG):
                        pds = [
                            pall.tile([P, c.DC], F32, tag=DTAGS[j], bufs=2,
                                      name=f"pd_{q}_{d}_{g}_{j}")
                            for j in range(c.MG)
                        ]
                        for s0 in range(0, c.SQ, SB):
                            wdt = wdpool.tile([P, SB, c.DC], BF16, tag="wd",
                                              name=f"wd_{q}_{d}_{g}_{s0}")
                            f0 = q * c.SQ + s0
                            nc.sync.dma_start(wdt, wd[d][:, f0:f0 + SB, :])
                            for si in range(SB):
                                s = s0 + si
                                for j in range(c.MG):
                                    m = g * c.MG + j
                                    nc.tensor.matmul(
                                        pds[j],
                                        x3[s][:, m * P:(m + 1) * P],
                                        wdt[:, si, :],
                                        start=(s == 0),
                                        stop=(s == c.SQ - 1))
                        for j in range(c.MG):
                            m = g * c.MG + j
                            if q == 0:
                                # seed with b_down on the first partial
                                nc.vector.tensor_add(
                                    yacc[:, m, dsl], pds[j], bdf[:, dsl])
                            elif q < c.FQ - 1:
                                nc.vector.tensor_add(
                                    yacc[:, m, dsl], pds[j], yacc[:, m, dsl])
                            else:
                                ot = outp.tile([P, c.DC], F32, tag="ot",
                                               name=f"ot_{d}_{g}_{j}")
                                nc.vector.tensor_add(
                                    ot, pds[j], yacc[:, m, dsl])
                                # ACT's HWDGE ring is idle during the down
                                # phase — keeps writeback off the wd/weight
                                # rings entirely
                                nc.scalar.dma_start(
                                    y[m * P:(m + 1) * P, dsl], ot)

    nc.compile()
    return nc


def _prep_weights(w, a, b):
    """Fold LoRA into base weight (float64 accumulate, bf16 round)."""
    weff = (w.astype(np.float64) + a.astype(np.float64) @ b.astype(np.float64))
    return weff.astype(np.float32)


def prep_inputs(inputs, cfg: Cfg):
    c = cfg
    x = np.asarray(inputs["x1"], np.float32).reshape(-1, c.D)
    n_tok = x.shape[0]
    assert n_tok == c.NT * c.N_CORES
    wg_e = _prep_weights(np.asarray(inputs["w_gate"], np.float32),
                         np.asarray(inputs["w_gate_lora_a"], np.float32),
                         np.asarray(inputs["w_gate_lora_b"], np.float32))
    wu_e = _prep_weights(np.asarray(inputs["w_up"], np.float32),
                         np.asarray(inputs["w_up_lora_a"], np.float32),
                         np.asarray(inputs["w_up_lora_b"], np.float32))
    wd_e = _prep_weights(np.asarray(inputs["w_down"], np.float32),
                         np.asarray(inputs["w_down_lora_a"], np.float32),
                         np.asarray(inputs["w_down_lora_b"], np.float32))
    # W[k_idx*P+kk, ft*P+ff] -> [ft, kk, k_idx, ff]
    wg_t = np.ascontiguousarray(
        wg_e.reshape(c.KC, P, c.NF, P).transpose(2, 1, 0, 3)).astype(NP_BF16)
    wu_t = np.ascontiguousarray(
        wu_e.reshape(c.KC, P, c.NF, P).transpose(2, 1, 0, 3)).astype(NP_BF16)
    # Wd[ft*P+ff, d*DC+dd] -> [d, ff, ft, dd]
    wd_t = np.ascontiguousarray(
        wd_e.reshape(c.NF, P, c.ND, c.DC).transpose(2, 1, 0, 3)).astype(NP_BF16)
    bg2 = np.ascontiguousarray(
        np.asarray(inputs["b_gate"], np.float32).reshape(c.NF, P).T)
    bu2 = np.ascontiguousarray(
        np.asarray(inputs["b_up"], np.float32).reshape(c.NF, P).T)
    bdf = np.ascontiguousarray(np.broadcast_to(
        np.asarray(inputs["b_down"], np.float32), (P, c.D)))
    in_maps = []
    for i in range(c.N_CORES):
        xs = x[i * c.NT:(i + 1) * c.NT]
        # [NT, D] -> [kk, k_idx, m]
        xt = np.ascontiguousarray(
            xs.T.reshape(c.KC, P, c.NT).transpose(1, 0, 2)).astype(NP_BF16)
        in_maps.append({
            "xt": xt,
            "wg": wg_t, "wu": wu_t, "wd": wd_t,
            "bg": bg2, "bu": bu2, "bd": bdf,
        })
    return in_maps


_CACHE = {}


def run(inputs, trace=False, trace_kwargs=None):
    cfg = Cfg()
    b, s, d = np.asarray(inputs["x1"]).shape
    in_maps = prep_inputs(inputs, cfg)
    key = "full"
    if key not in _CACHE:
        _CACHE[key] = build_bass(cfg)
    nc = _CACHE[key]
    res = run_bass_kernel_spmd(
        nc, in_maps, list(range(cfg.N_CORES)),
        trace=trace, **(trace_kwargs or {}))
    y = np.concatenate([res.results[i]["y"] for i in range(cfg.N_CORES)], axis=0)
    return y.reshape(b, s, d).astype(np.float32), res


def kernel(**inputs) -> np.ndarray:
    out, _ = run(inputs, trace=False)
    return out


# revision 13
# speedup vs baseline: 1.2846x; 1.0035x over previous
"""Trainium2 Bass kernel for a LoRA-augmented relu-gated MLP.

Math (per reference):
    y1 = x @ w_gate + b_gate + (x @ Ag) @ Bg
    y2 = x @ w_up   + b_up   + (x @ Au) @ Bu
    x3 = relu(y1) * y2
    y3 = x3 @ w_down + b_down + (x3 @ Ad) @ Bd

Strategy:
  * Host folds every LoRA pair into its base matrix (W_eff = W + A@B in
    float64) and rounds weights + activations to bf16 so the device kernel
    is a plain gated MLP running bf16 matmuls with f32 PSUM accumulation.
    bf16 stationary operands enable the PE's Fast Weight Load path, which
    hides the per-matmul LDWEIGHTS under the 512-column stream (fp32
    weights load at half rate and serialize ~60ns/matmul on top).
  * Data parallel over the 8 NeuronCores: 8192 tokens -> 1024 per core,
    every core holds the full (folded) weights.
  * Per core the MLP is computed in f-quarters: gate/up produce x3T
    stripes [128f, NT] (bf16) in SBUF; the down projection consumes them
    as stationary operands and accumulates partial y3 into an SBUF-resident
    f32 accumulator; b_down is added on the first quarter's eviction. The
    last quarter streams finished y chunks straight to DRAM.
"""

import sys
import types

import numpy as np

# The trimmed container's `antenv` lacks `axon_hooks`; bass_utils imports it
# unconditionally when tracing is requested (e.g. BASS_TRACE=1). Provide the
# degraded no-hook module so tracing falls back gracefully instead of crashing.
try:
    import antenv.axon_hooks  # noqa: F401
except ImportError:
    _m = types.ModuleType("antenv.axon_hooks")
    _m._hook = None
    _m.set_axon_ntff_profile_hook = lambda h: setattr(_m, "_hook", h)
    _m.get_axon_ntff_profile_hook = lambda: _m._hook
    sys.modules["antenv.axon_hooks"] = _m

import concourse.bacc as bacc
import concourse.bass as bass
import concourse.mybir as mybir
import concourse.tile as tile
from concourse.bass_utils import run_bass_kernel_spmd

P = 128
F32 = mybir.dt.float32
BF16 = mybir.dt.bfloat16
NP_BF16 = mybir.dt.np(mybir.dt.bfloat16)
AF = mybir.ActivationFunctionType
ALU = mybir.AluOpType


class Cfg:
    def __init__(self, nt=1024, d=2048, f=8192, fq=4, n_cores=8):
        assert nt % P == 0 and d % P == 0 and f % P == 0
        self.NT = nt          # tokens per core
        self.D = d            # model dim
        self.F = f            # ffn dim
        self.KC = d // P      # contraction chunks for gate/up
        self.NF = f // P      # f-tiles
        self.FQ = fq          # f quarters (x3T resident per quarter)
        assert self.NF % fq == 0
        self.SQ = self.NF // fq
        self.MH = min(512, nt)          # moving-dim chunk for gate/up
        self.NMH = nt // self.MH
        self.DC = min(512, d)           # down-proj d chunk
        self.ND = d // self.DC
        self.NM = nt // P               # token chunks of 128
        self.MG = 4                     # psum group size for down-proj
        self.NMG = self.NM // self.MG
        self.N_CORES = n_cores


def build_bass(cfg: Cfg):
    """Builds the per-core Bass program (same program on all cores)."""
    c = cfg
    nc = bacc.Bacc("TRN2", target_bir_lowering=False, debug=False,
                   num_swdge_queues=4)

    xt = nc.dram_tensor("xt", [P, c.NMH, c.KC, c.MH], BF16, kind="ExternalInput")
    wg = nc.dram_tensor("wg", [c.NF, P, c.KC, P], BF16, kind="ExternalInput")
    wu = nc.dram_tensor("wu", [c.NF, P, c.KC, P], BF16, kind="ExternalInput")
    wd = nc.dram_tensor("wd", [c.ND, P, c.NF, c.DC], BF16, kind="ExternalInput")
    bg = nc.dram_tensor("bg", [P, c.NF], F32, kind="ExternalInput")
    bu = nc.dram_tensor("bu", [P, c.NF], F32, kind="ExternalInput")
    bd = nc.dram_tensor("bd", [P, c.D], F32, kind="ExternalInput")
    y = nc.dram_tensor("y", [c.NT, c.D], F32, kind="ExternalOutput")

    with tile.TileContext(nc) as tc:
        with (
            tc.tile_pool(name="consts", bufs=1) as consts,
            tc.tile_pool(name="wpool", bufs=4) as wpool,
            tc.tile_pool(name="wdpool", bufs=3) as wdpool,
            tc.tile_pool(name="xTp", bufs=1) as xTp,
            tc.tile_pool(name="x3p", bufs=1) as x3p,
            tc.tile_pool(name="yp", bufs=1) as yp,
            tc.tile_pool(name="actp", bufs=2) as actp,
            tc.tile_pool(name="outp", bufs=6) as outp,
            tc.tile_pool(name="pall", bufs=1, space="PSUM") as pall,
        ):
            # weight stream + y writeback ride SWDGE (gpsimd) queues; the
            # SP HWDGE ring carries xT and the down-proj wd batches so
            # neither stream head-of-line-blocks the other. The first two
            # f-tiles' weights go on the faster-spinning sync ring instead
            # (SWDGE's Q7 takes several us to emit its first descriptors).
            def load_w(ft, eng=None):
                wgt = wpool.tile([P, c.KC, P], BF16, tag="w", name=f"wg{ft}")
                (eng or nc.gpsimd).dma_start(wgt, wg[ft])
                wut = wpool.tile([P, c.KC, P], BF16, tag="w", name=f"wu{ft}")
                (eng or nc.gpsimd).dma_start(wut, wu[ft])
                return wgt, wut

            xT = xTp.tile([P, c.NMH, c.KC, c.MH], BF16, name="xT")
            # staircase batch sizes: small first batches so the PE starts
            # (and the HAM clock warms) as early as possible, large later
            # batches to keep descriptor counts low. The [h, k, m] layout
            # keeps each batch contiguous per partition (>=2KB lines).
            ladders = {0: [1, 1, 2, 4, 8], 1: [8, 8]}
            with tc.high_priority():
                pend = {0: load_w(0, nc.sync)}
                for h in range(c.NMH):
                    k0 = 0
                    for kb in ladders.get(h, [c.KC]):
                        nc.sync.dma_start(xT[:, h, k0:k0 + kb, :],
                                          xt[:, h, k0:k0 + kb, :])
                        k0 += kb
                    if h == 0:
                        pend[1] = load_w(1, nc.sync)
                bgt = consts.tile([P, c.NF], F32, name="bgt")
                nc.gpsimd.dma_start(bgt, bg[:, :])
                but = consts.tile([P, c.NF], F32, name="but")
                nc.gpsimd.dma_start(but, bu[:, :])
            bdf = consts.tile([P, c.D], F32, name="bdf")
            nc.gpsimd.dma_start(bdf, bd[:, :])

            # y accumulator, SBUF-resident across the 4 f-quarters
            yacc = yp.tile([P, c.NM, c.D], F32, name="yacc")

            DTAGS = ["p1", "p2", "pd0", "pd1"]
            for q in range(c.FQ):
                # ---- gate/up projections for this f-quarter ----
                x3 = [
                    x3p.tile([P, c.NT], BF16, tag=f"s{s}", name=f"x3_{q}_{s}")
                    for s in range(c.SQ)
                ]
                for s in range(c.SQ):
                    ft = q * c.SQ + s
                    wgt, wut = pend.pop(ft) if ft in pend else load_w(ft)
                    if ft + 2 < c.NF and ft + 2 not in pend:
                        pend[ft + 2] = load_w(ft + 2)
                    for h in range(c.NMH):
                        msl = slice(h * c.MH, (h + 1) * c.MH)
                        p1 = pall.tile([P, c.MH], F32, tag="p1", bufs=2,
                                       name=f"p1_{ft}_{h}")
                        p2 = pall.tile([P, c.MH], F32, tag="p2", bufs=2,
                                       name=f"p2_{ft}_{h}")
                        for k in range(c.KC):
                            nc.tensor.matmul(
                                p1, wgt[:, k, :],
                                xT[:, h, k, :],
                                start=(k == 0), stop=(k == c.KC - 1))
                        for k in range(c.KC):
                            nc.tensor.matmul(
                                p2, wut[:, k, :],
                                xT[:, h, k, :],
                                start=(k == 0), stop=(k == c.KC - 1))
                        t1 = actp.tile([P, c.MH], F32, tag="t1", name=f"t1_{ft}_{h}")
                        nc.scalar.activation(t1, p1, AF.Relu, bias=bgt[:, ft:ft + 1])
                        # x3 = (p2 + b_up) * relu(p1 + b_gate)
                        nc.vector.scalar_tensor_tensor(
                            x3[s][:, msl], p2, but[:, ft:ft + 1], t1,
                            op0=ALU.add, op1=ALU.mult)
                # ---- down projection partials for this f-quarter ----
                SB = 8  # wd stripes per DMA batch
                for d in range(c.ND):
                    dsl = slice(d * c.DC, (d + 1) * c.DC)
                    for g in range(c.NMG):
                        pds = [
                            pall.tile([P, c.DC], F32, tag=DTAGS[j], bufs=2,
                                      name=f"pd_{q}_{d}_{g}_{j}")
                            for j in range(c.MG)
                        ]
                        for s0 in range(0, c.SQ, SB):
                            wdt = wdpool.tile([P, SB, c.DC], BF16, tag="wd",
                                              name=f"wd_{q}_{d}_{g}_{s0}")
                            f0 = q * c.SQ + s0
                            nc.sync.dma_start(wdt, wd[d][:, f0:f0 + SB, :])
                            for si in range(SB):
                                s = s0 + si
                                for j in range(c.MG):
                                    m = g * c.MG + j
                                    nc.tensor.matmul(
                                        pds[j],
                                        x3[s][:, m * P:(m + 1) * P],
                                        wdt[:, si, :],
                                        start=(s == 0),
                                        stop=(s == c.SQ - 1))
                        for j in range(c.MG):
                            m = g * c.MG + j
                            if q == 0:
                                # seed with b_down on the first partial
                                nc.vector.tensor_add(
                                    yacc[:, m, dsl], pds[j], bdf[:, dsl])
                            elif q < c.FQ - 1:
                                nc.vector.tensor_add(
                                    yacc[:, m, dsl], pds[j], yacc[:, m, dsl])
                            else:
                                ot = outp.tile([P, c.DC], F32, tag="ot",
                                               name=f"ot_{d}_{g}_{j}")
                                nc.vector.tensor_add(
                                    ot, pds[j], yacc[:, m, dsl])
                                # ACT's HWDGE ring is idle during the down
                                # phase — keeps writeback off the wd/weight
                                # rings entirely
                                nc.scalar.dma_start(
                                    y[m * P:(m + 1) * P, dsl], ot)

    nc.compile()
    return nc


def _prep_weights(w, a, b):
    """Fold LoRA into base weight (float64 accumulate, bf16 round)."""
    weff = (w.astype(np.float64) + a.astype(np.float64) @ b.astype(np.float64))
    return weff.astype(np.float32)


def prep_inputs(inputs, cfg: Cfg):
    c = cfg
    x = np.asarray(inputs["x1"], np.float32).reshape(-1, c.D)
    n_tok = x.shape[0]
    assert n_tok == c.NT * c.N_CORES
    wg_e = _prep_weights(np.asarray(inputs["w_gate"], np.float32),
                         np.asarray(inputs["w_gate_lora_a"], np.float32),
                         np.asarray(inputs["w_gate_lora_b"], np.float32))
    wu_e = _prep_weights(np.asarray(inputs["w_up"], np.float32),
                         np.asarray(inputs["w_up_lora_a"], np.float32),
                         np.asarray(inputs["w_up_lora_b"], np.float32))
    wd_e = _prep_weights(np.asarray(inputs["w_down"], np.float32),
                         np.asarray(inputs["w_down_lora_a"], np.float32),
                         np.asarray(inputs["w_down_lora_b"], np.float32))
    # W[k_idx*P+kk, ft*P+ff] -> [ft, kk, k_idx, ff]
    wg_t = np.ascontiguousarray(
        wg_e.reshape(c.KC, P, c.NF, P).transpose(2, 1, 0, 3)).astype(NP_BF16)
    wu_t = np.ascontiguousarray(
        wu_e.reshape(c.KC, P, c.NF, P).transpose(2, 1, 0, 3)).astype(NP_BF16)
    # Wd[ft*P+ff, d*DC+dd] -> [d, ff, ft, dd]
    wd_t = np.ascontiguousarray(
        wd_e.reshape(c.NF, P, c.ND, c.DC).transpose(2, 1, 0, 3)).astype(NP_BF16)
    bg2 = np.ascontiguousarray(
        np.asarray(inputs["b_gate"], np.float32).reshape(c.NF, P).T)
    bu2 = np.ascontiguousarray(
        np.asarray(inputs["b_up"], np.float32).reshape(c.NF, P).T)
    bdf = np.ascontiguousarray(np.broadcast_to(
        np.asarray(inputs["b_down"], np.float32), (P, c.D)))
    in_maps = []
    for i in range(c.N_CORES):
        xs = x[i * c.NT:(i + 1) * c.NT]
        # [NT, D] -> [kk, h, k_idx, m']
        xt = np.ascontiguousarray(
            xs.T.reshape(c.KC, P, c.NMH, c.MH).transpose(1, 2, 0, 3)
        ).astype(NP_BF16)
        in_maps.append({
            "xt": xt,
            "wg": wg_t, "wu": wu_t, "wd": wd_t,
            "bg": bg2, "bu": bu2, "bd": bdf,
        })
    return in_maps


_CACHE = {}


def run(inputs, trace=False, trace_kwargs=None):
    cfg = Cfg()
    b, s, d = np.asarray(inputs["x1"]).shape
    in_maps = prep_inputs(inputs, cfg)
    key = "full"
    if key not in _CACHE:
        _CACHE[key] = build_bass(cfg)
    nc = _CACHE[key]
    res = run_bass_kernel_spmd(
        nc, in_maps, list(range(cfg.N_CORES)),
        trace=trace, **(trace_kwargs or {}))
    y = np.concatenate([res.results[i]["y"] for i in range(cfg.N_CORES)], axis=0)
    return y.reshape(b, s, d).astype(np.float32), res


def kernel(**inputs) -> np.ndarray:
    out, _ = run(inputs, trace=False)
    return out
